# revision 7
# baseline (speedup 1.0000x reference)
"""GQA multi-head attention (B=1, S=4096, E=2048, H=16, HK=4, D=128) on 8 trn2
NeuronCores.

Sharding: tensor-parallel over query heads — 2 q-heads per core, each core
computes the kv head its q-heads attend to (each kv head is replicated on the
2 cores that need it). Each core produces a partial output y_c = attn_c @ Wo_c
and the host sums the 8 partials (the "all-reduce" happens host-side during
unsharding, so the device program needs no collectives).

Device-side dataflow per core (all matmul inputs fp16, accumulation fp32):
  xT [E,S] -> qT [D,h,S], kT [D,S] (transposed projections), v [S,D]
  scoresT[t,sq] = kT_chunk.T @ qT  (t-chunk of 128 on partitions)
  pT = exp(scoresT/sqrt(D)) via ACT, fp16
  outT[d,sq] += v_chunk.T-less matmul: lhsT=v[t,d], rhs=pT[t,sq] accumulated
  rowsums via DVE adds over t-chunks + GPSIMD partition_all_reduce
  attnT = outT * (1/rowsum)  -> o_proj: y[s,e] = attnT.T @ WoT
"""
import math
import numpy as np
from contextlib import ExitStack

import concourse.bass as bass
import concourse.mybir as mybir
from concourse import tile
from concourse import bass_utils

B, S, E = 1, 4096, 2048
H, HK, D = 16, 4, 128
N_CORES = 8
HPC = H // N_CORES          # q heads per core
QDIM = HPC * D              # 256
EC = E // 128               # e-chunks
SB = 512                    # s/sq block
NSB = S // SB
TC = S // 128               # t-chunks
SCALE = 1.0 / math.sqrt(D)
FP16 = mybir.dt.float16
FP32 = mybir.dt.float32


def _split_sync_waits(nc, cap=1):
    """This container's walrus build rejects instructions carrying more than
    ~1 sync-wait (codegen 'Too many sync wait commands'). Post-pass over the
    scheduled BIR: for any instruction with >cap waits, hoist the excess onto
    same-engine NOPs inserted immediately before it (same block, so per-engine
    program order — and therefore semantics — is preserved)."""
    n = 0
    for fn in nc.m.functions:
        for blk in fn.blocks:
            il = blk.instructions
            i = 0
            while i < len(il):
                inst = il[i]
                si = getattr(inst, "sync_info", None)
                if si is not None and len(si.on_wait) > cap:
                    waits = list(si.on_wait)
                    si.on_wait = waits[-cap:]
                    extras = []
                    for w in waits[:-cap]:
                        nop = mybir.InstNoOp(name=f"I-waitfix-{n}", ins=[], outs=[])
                        n += 1
                        nop.engine = inst.engine
                        nop.sync_info = mybir.SyncInfo(on_wait=[w], on_update=[])
                        extras.append(nop)
                    il[i:i] = extras
                    i += len(extras)
                i += 1
    return n


def build_bass():
    nc = bass.Bass("TRN2", target_bir_lowering=False, debug=False,
                   num_devices=N_CORES)
    xT = nc.dram_tensor("xT", [E, S], FP16, kind="ExternalInput").ap()
    wq = nc.dram_tensor("wq", [E, QDIM], FP16, kind="ExternalInput").ap()
    wk = nc.dram_tensor("wk", [E, D], FP16, kind="ExternalInput").ap()
    wv = nc.dram_tensor("wv", [E, D], FP16, kind="ExternalInput").ap()
    wo = nc.dram_tensor("wo", [QDIM, E], FP16, kind="ExternalInput").ap()
    y = nc.dram_tensor("y", [S, E], FP16, kind="ExternalOutput").ap()

    with tile.TileContext(nc) as tc, ExitStack() as ctx:
        wpool = ctx.enter_context(tc.tile_pool(name="wpool", bufs=1))
        big = ctx.enter_context(tc.tile_pool(name="big", bufs=1))

        wq_sb = wpool.tile([128, EC, QDIM], FP16)
        wk_sb = wpool.tile([128, EC, D], FP16)
        wv_sb = wpool.tile([128, EC, D], FP16)
        wo_sb = wpool.tile([128, HPC, E], FP16)
        for ec in range(EC):
            nc.sync.dma_start(wq_sb[:, ec, :], wq[ec * 128:(ec + 1) * 128, :])
            nc.sync.dma_start(wk_sb[:, ec, :], wk[ec * 128:(ec + 1) * 128, :])
            nc.sync.dma_start(wv_sb[:, ec, :], wv[ec * 128:(ec + 1) * 128, :])
        for h in range(HPC):
            nc.sync.dma_start(wo_sb[:, h, :], wo[h * 128:(h + 1) * 128, :])

        qT_sb = big.tile([128, HPC, S], FP16)   # [d, h, s]
        kT_sb = big.tile([128, S], FP16)        # [d, t]
        v_sb = big.tile([128, S], FP16)         # [t%128, tc*128 + d]
        aT_sb = big.tile([128, HPC, S], FP16)   # [d, h, s] normalized attn out
        ones_sb = big.tile([128, 128], FP32)    # for partition broadcast-sum
        nc.vector.memset(ones_sb[:], 1.0)

        # ---- Phase 1: projections ----
        with ExitStack() as p1:
            xpool = p1.enter_context(tc.tile_pool(name="xpool", bufs=24))
            pj_ps = p1.enter_context(tc.tile_pool(name="pj_ps", bufs=2, space="PSUM"))
            for sb in range(NSB):
                xts = []
                for ec in range(EC):
                    xt = xpool.tile([128, SB], FP16, name=f"xt_{sb}_{ec}", tag="xt")
                    nc.sync.dma_start(
                        xt[:], xT[ec * 128:(ec + 1) * 128, sb * SB:(sb + 1) * SB])
                    xts.append(xt)
                for ic in range(HPC):
                    q_ps = pj_ps.tile([128, SB], FP32, name=f"qps_{sb}_{ic}", tag="qps")
                    for ec in range(EC):
                        nc.tensor.matmul(
                            q_ps[:],
                            wq_sb[:, ec, ic * 128:(ic + 1) * 128],
                            xts[ec][:],
                            start=(ec == 0), stop=(ec == EC - 1))
                    nc.vector.tensor_copy(
                        qT_sb[:, ic, sb * SB:(sb + 1) * SB], q_ps[:])
                k_ps = pj_ps.tile([128, SB], FP32, name=f"kps_{sb}", tag="kps")
                for ec in range(EC):
                    nc.tensor.matmul(k_ps[:], wk_sb[:, ec, :], xts[ec][:],
                                     start=(ec == 0), stop=(ec == EC - 1))
                nc.vector.tensor_copy(kT_sb[:, sb * SB:(sb + 1) * SB], k_ps[:])
                v_ps = pj_ps.tile([128, SB], FP32, name=f"vps_{sb}", tag="vps")
                for j in range(SB // 128):
                    for ec in range(EC):
                        nc.tensor.matmul(
                            v_ps[:, j * 128:(j + 1) * 128],
                            xts[ec][:, j * 128:(j + 1) * 128],
                            wv_sb[:, ec, :],
                            start=(ec == 0), stop=(ec == EC - 1))
                nc.scalar.copy(v_sb[:, sb * SB:(sb + 1) * SB], v_ps[:])

        # ---- Phase 2: attention ----
        with ExitStack() as p2:
            s_psp = p2.enter_context(tc.tile_pool(name="s_psp", bufs=2, space="PSUM"))
            o_psp = p2.enter_context(tc.tile_pool(name="o_psp", bufs=2, space="PSUM"))
            ptp = p2.enter_context(tc.tile_pool(name="ptp", bufs=4))
            accp = p2.enter_context(tc.tile_pool(name="accp", bufs=2))
            for h in range(HPC):
                for qb in range(NSB):
                    o_ps = o_psp.tile([128, SB], FP32, name=f"ops_{h}_{qb}", tag="ops")
                    acc2 = accp.tile([128, 2 * SB], FP32, name=f"acc_{h}_{qb}", tag="acc")
                    for tp in range(TC // 2):
                        s_ps = s_psp.tile([128, 2 * SB], FP32,
                                          name=f"sps_{h}_{qb}_{tp}", tag="sps")
                        for hf in range(2):
                            t = tp * 2 + hf
                            nc.tensor.matmul(
                                s_ps[:, hf * SB:(hf + 1) * SB],
                                kT_sb[:, t * 128:(t + 1) * 128],
                                qT_sb[:, h, qb * SB:(qb + 1) * SB],
                                start=True, stop=True)
                        pt = ptp.tile([128, 2 * SB], FP16,
                                      name=f"pt_{h}_{qb}_{tp}", tag="pt")
                        nc.scalar.activation(
                            pt[:], s_ps[:],
                            mybir.ActivationFunctionType.Exp, scale=SCALE)
                        for hf in range(2):
                            t = tp * 2 + hf
                            nc.tensor.matmul(
                                o_ps[:],
                                v_sb[:, t * 128:(t + 1) * 128],
                                pt[:, hf * SB:(hf + 1) * SB],
                                start=(t == 0), stop=(t == TC - 1))
                        if tp == 0:
                            nc.vector.tensor_copy(acc2[:], pt[:])
                        else:
                            nc.vector.tensor_add(acc2[:], acc2[:], pt[:])
                    sums = accp.tile([128, SB], FP32, name=f"sums_{h}_{qb}", tag="sums")
                    nc.vector.tensor_add(sums[:], acc2[:, 0:SB], acc2[:, SB:2 * SB])
                    # partition-sum + broadcast in one matmul: ones.T @ sums
                    sums_ps = o_psp.tile([128, SB], FP32,
                                         name=f"sums_ps_{h}_{qb}", tag="sums_ps")
                    nc.tensor.matmul(sums_ps[:], ones_sb[:], sums[:],
                                     start=True, stop=True)
                    recip = accp.tile([128, SB], FP32,
                                      name=f"recip_{h}_{qb}", tag="recip")
                    nc.vector.reciprocal(recip[:], sums_ps[:])
                    nc.vector.tensor_mul(
                        aT_sb[:, h, qb * SB:(qb + 1) * SB], o_ps[:], recip[:])

        # ---- Phase 3: output projection ----
        with ExitStack() as p3:
            y_psp = p3.enter_context(tc.tile_pool(name="y_psp", bufs=2, space="PSUM"))
            y_sbp = p3.enter_context(tc.tile_pool(name="y_sbp", bufs=3))
            for sc in range(S // 128):
                y_ps = y_psp.tile([128, E], FP32, name=f"yps_{sc}", tag="yps")
                for h in range(HPC):
                    for eb in range(E // SB):
                        nc.tensor.matmul(
                            y_ps[:, eb * SB:(eb + 1) * SB],
                            aT_sb[:, h, sc * 128:(sc + 1) * 128],
                            wo_sb[:, h, eb * SB:(eb + 1) * SB],
                            start=(h == 0), stop=(h == HPC - 1))
                y_t = y_sbp.tile([128, E], FP16, name=f"ysb_{sc}", tag="ysb")
                nc.scalar.copy(y_t[:], y_ps[:])
                nc.sync.dma_start(y[sc * 128:(sc + 1) * 128, :], y_t[:])

    _split_sync_waits(nc)
    return nc


def make_in_maps(x, Wq, Wk, Wv, Wo):
    """Host-side sharding: transpose/cast to fp16, slice weights per core."""
    x = np.asarray(x, dtype=np.float32).reshape(S, E)
    xT = np.ascontiguousarray(x.T).astype(np.float16)
    WqT = np.ascontiguousarray(np.asarray(Wq, dtype=np.float32).T).astype(np.float16)
    WkT = np.ascontiguousarray(np.asarray(Wk, dtype=np.float32).T).astype(np.float16)
    WvT = np.ascontiguousarray(np.asarray(Wv, dtype=np.float32).T).astype(np.float16)
    WoT = np.ascontiguousarray(np.asarray(Wo, dtype=np.float32).T).astype(np.float16)
    in_maps = []
    for c in range(N_CORES):
        g = (c * HPC) // (H // HK)      # kv head for this core's q heads
        in_maps.append({
            "xT": xT,
            "wq": np.ascontiguousarray(WqT[:, c * QDIM:(c + 1) * QDIM]),
            "wk": np.ascontiguousarray(WkT[:, g * D:(g + 1) * D]),
            "wv": np.ascontiguousarray(WvT[:, g * D:(g + 1) * D]),
            "wo": np.ascontiguousarray(WoT[c * QDIM:(c + 1) * QDIM, :]),
        })
    return in_maps


_NC_CACHE = None


def get_nc():
    global _NC_CACHE
    if _NC_CACHE is None:
        _NC_CACHE = build_bass()
    return _NC_CACHE


def kernel(x, Wq, Wk, Wv, Wo):
    nc = get_nc()
    in_maps = make_in_maps(x, Wq, Wk, Wv, Wo)
    res = bass_utils.run_bass_kernel_spmd(
        nc, in_maps, core_ids=list(range(N_CORES)))
    out = np.zeros((S, E), dtype=np.float32)
    for r in res.results:
        out += r["y"].astype(np.float32)
    return out.reshape(B, S, E)
